# revision 16
# baseline (speedup 1.0000x reference)
"""Trainium2 Bass kernel for nn_CustomParameterTransform (scatter_memory).

Reference semantics: coord_v [256, 30] holds 10 (x, y, mass) triplets per
sample. Each triplet maps to integer grid indices (x_i, y_i, m_i); a one-hot
volume z [B, 16, 128, 128] is scattered (z[b, m, y, x] = 1) and the output is
concat(1-z, z) over the channel axis -> [256, 32, 128, 128] f32 (512 MB).

Strategy (8 NeuronCores, 32 samples/core, no cross-core comm): the output is
almost entirely constant, so the kernel is a pure HBM write stream (64 MB
per core) plus 640 one-element fixups per core.

Per-core output layout (host re-assembles): ones region [32 samples x 1 MB]
(the 1-z half: 1.0 except scatter points), then zeros region (the z half).

Fill plan, 2048 32KB blocks per core:
  - 52 MB static HWDGE fills (sync: most of ones; scalar: most of zeros)
    from constant SBUF tiles - every DMA engine gets exactly 104 blocks.
  - 12.5 MB early SWDGE indirect fills (gpsimd) whose 32KB blocks are
    addressed by a host-supplied per-core index tensor; descriptor slot
    rows map to fixed DMA engines (rows [4q,4q+4) -> engine (2q)%16 for
    q<16, else (2(q-16)+1)%16 - measured), and out-of-bounds indices are
    silently skipped, so the host shapes per-engine bytes per core.
  On this box one specific engine per even-numbered physical core
  intermittently runs ~20% slow (nc0/nc4 -> engine position 15, nc2/nc6 ->
  position 0; jax cores map to nc (4,5,6,7,2,3,0,1)). 104 blocks is the
  optimal share for a slow engine (104/21.3GB/s ~= 129.5/26.5GB/s), so on
  risky cores the host gives that engine no SWDGE blocks at all and spreads
  them over the other 15 engines; on healthy cores the layout is flat.
  Equalized finish ~157us vs ~197us for a flat layout with a slow engine,
  and the skew costs nothing when the engine is healthy.
  - 640 scatter fixups as 6 indirect-DMA columns (ones cols write 0.0,
    zeros cols 1.0), each depending only on the fills covering its
    address range so the last one fires right after the final fill.
"""

import numpy as np

B = 256
NSRC = 10
NMC = 16
L = 128
NCORES = 8
BL = B // NCORES            # 32 samples per core
PLANE = L * L               # 16384
HALF = NMC * PLANE          # 262144 elements per sample half (1 MB)
REGION = BL * HALF          # 8388608 elements per region (32 MB)
OUT_ELEMS = 2 * REGION      # 16777216 per core (64 MB)

BLK = 8192                  # elements per 32 KB fill block
NBLOCKS = OUT_ELEMS // BLK  # 2048
ZBASE = REGION // BLK       # first zeros-region block (1024)
HEAD_UNIT = 1024            # elements per 4 KB head-fill unit
HEAD_BLOCKS = 16            # head covers 512 KB = blocks 0..15

# SWDGE-shaped block ranges (absolute block ids). The zeros region is
# filled FIRST (its head bootstraps the stream; its memset runs first) and
# the ones region LAST, so five of the six scatter columns fire mid-stream
# and only ones-col2 trails the final fill.
ONES_SH_LO, ONES_SH_HI = 0, 208                  # 208 blocks
ZEROS_SH_LO, ZEROS_SH_HI = ZBASE + 16, ZBASE + 208   # 192 blocks
HEAD_START = ZBASE          # zeros head: blocks 1024..1039
N_SH_ONES = 2               # shaped instructions per region
N_SH_ZEROS = 2
N_SW_FILLS = 1 + N_SH_ONES + N_SH_ZEROS          # 5 SWDGE fill instrs

# Static HWDGE fills (start block, nblocks), in issue order per queue:
# both queues stream zeros first, then ones; the ones tails are the very
# last fills so only scatter col 2 depends on the stream end.
SYNC_FILLS = [(1232, 64), (1296, 64), (1360, 128), (1488, 128), (1616, 128),
              (720, 128), (848, 128), (976, 48)]
SCAL_FILLS = [(1744, 64), (1808, 64), (1872, 128), (2000, 48),
              (208, 128), (336, 128), (464, 128), (592, 128)]

# Scatter columns: (region, first point, npoints). The last ones column is
# tiny (24 points) and confined to the rows of engines 2-4 so its
# completion (the only one after the final fill) costs ~3 engine-sem
# updates instead of 16.
SCAT_COLS = [("z", 0, 128), ("z", 128, 128), ("z", 256, 64),
             ("o", 0, 128), ("o", 128, 128), ("o", 256, 40), ("o", 296, 24)]
N_SCAT = len(SCAT_COLS)
NCOL = N_SW_FILLS + N_SCAT  # offs input columns

OOB = np.int32(0x7FFFFFF)

# jax core index -> engine position to starve (measured; absent = flat).
STARVE_POS = {0: 15, 2: 0, 4: 0, 6: 15}

_CACHE = {}


def _rows_of_pos(p):
    """The 8 descriptor-slot rows served by DMA engine position p."""
    if p % 2 == 0:
        q = p // 2
        return list(range(4 * q, 4 * q + 4)) + \
            list(range(4 * (q + 8), 4 * (q + 8) + 4))
    q = (p - 1) // 2
    return [64 + r for r in range(4 * q, 4 * q + 4)] + \
        [64 + r for r in range(4 * (q + 8), 4 * (q + 8) + 4)]


_POS_ROWS = [_rows_of_pos(p) for p in range(16)]


def _shaped_cols(starve_pos, block_ids, n_instr):
    """Assign shaped blocks to (instr, row) slots, engine-balanced.

    Returns int64 [128, n_instr] block ids (-1 = OOB slot)."""
    nb = len(block_ids)
    quota = [0] * 16
    if starve_pos is None:
        for p in range(16):
            quota[p] = nb // 16
        for p in range(nb % 16):
            quota[p] += 1
    else:
        healthy = [p for p in range(16) if p != starve_pos]
        for i, p in enumerate(healthy):
            quota[p] = nb // 15 + (1 if i < nb % 15 else 0)
    cols = np.full((128, n_instr), -1, dtype=np.int64)
    pos_it = 0
    for p in range(16):
        rows = _POS_ROWS[p]
        q = quota[p]
        assert q <= 8 * n_instr, (p, q)
        left = q
        for k in range(n_instr):
            t = min(8, left)
            for r in rows[:t]:
                cols[r, k] = block_ids[pos_it]
                pos_it += 1
            left -= t
    assert pos_it == nb, (pos_it, nb)
    return cols


def _class_layouts():
    out = {}
    for sp in (None, 0, 15):
        oc = _shaped_cols(sp, list(range(ONES_SH_LO, ONES_SH_HI)), N_SH_ONES)
        zc = _shaped_cols(sp, list(range(ZEROS_SH_LO, ZEROS_SH_HI)),
                          N_SH_ZEROS)
        out[sp] = (oc, zc)
    return out


_LAYOUTS = _class_layouts()


def _col_bmax(j):
    """Worst-case largest region-relative block id touched by scatter
    column j: points are exactly 10 per sample, address-sorted."""
    _, first, npts = SCAT_COLS[j]
    last_pt = first + npts - 1
    return 32 * (last_pt // 10 + 1) - 1


# Rows for the final tiny ones column: engines 2..4 (never the slow one).
_TINY_ROWS = sorted(_rows_of_pos(2) + _rows_of_pos(3) + _rows_of_pos(4))


def _build_nc():
    import concourse.bass as bass
    import concourse.tile as tile
    from concourse import bacc, mybir
    from concourse.tile_rust import add_dep_helper

    import types as _types
    from concourse.vector_clock import ScopedClock

    nc = bacc.Bacc("TRN2", target_bir_lowering=False, debug=False,
                   num_devices=NCORES)

    def _light_drain_and_barrier(self, tick_clock, wait_clock):
        """Replaces TileContext._drain_and_barrier. The stock epilogue is
        drain + two all-engine EVSEM butterfly barriers around the sem
        clear (~9 us). Requirements at kernel end: (1) all DMA completions
        observed, (2) sems cleared for NEFF re-execution, (3) the clear
        after every engine's last sem use. (1) is the sync drain's
        global-clock waits; (3) is a counting-sem join; (2) the ranged
        clear. The second barrier is unnecessary: a re-execution cannot
        start until every engine - including the clearing gpsimd - has
        ended."""
        nc_ = self.nc
        drain_inst = nc_.sync.drain()
        wait_clock.add_sem_waits(
            drain_inst.ins, ScopedClock({None: tick_clock.global_clock}))
        join = nc_.alloc_semaphore("tail_join")
        for eng in nc_.engines.values():
            if eng is not nc_.gpsimd:
                eng.sem_inc(join, 1)
        n_other = len(nc_.engines) - 1
        nc_.gpsimd.wait_ge(join, n_other)
        popped = nc_._tile_sem_poison_stack.pop()
        assert popped == self._sem_poison
        sems = list(self.sems.allocated().values())
        nc_.clear_and_free_semaphores(sems + [join])

    offs = nc.dram_tensor("offs", [128, NCOL], mybir.dt.int32,
                          kind="ExternalInput").ap()
    out = nc.dram_tensor("out", [OUT_ELEMS], mybir.dt.float32,
                         kind="ExternalOutput").ap()

    with tile.TileContext(nc) as tc:
        tc._drain_and_barrier = _types.MethodType(_light_drain_and_barrier, tc)
        with tc.tile_pool(name="src", bufs=1) as pool:
            zeros_t = pool.tile([128, BLK], mybir.dt.float32)
            ones_t = pool.tile([128, BLK], mybir.dt.float32)
            # gpsimd memset is much faster than DVE's; slice the zeros tile
            # column-wise so the head fill and the 2MB first statics start
            # as early as possible. ones_t (DVE, slow) is only read in the
            # second half of the stream.
            ot = pool.tile([128, NCOL], mybir.dt.int32)
            nc.sync.dma_start(ot[:, :], offs[:, :])

            def swfill(j, in_ap, unit, bound):
                view = bass.AP(out.tensor, 0, [[unit, 1], [1, unit]],
                               dep_tracking_offset=j * BLK)
                return nc.gpsimd.indirect_dma_start(
                    out=view,
                    out_offset=bass.IndirectOffsetOnAxis(
                        ap=ot[:, j:j + 1], axis=0),
                    in_=in_ap, in_offset=None,
                    bounds_check=bound, oob_is_err=False)

            sw = [None] * N_SW_FILLS
            nc.gpsimd.memset(zeros_t[:, 0:HEAD_UNIT], 0.0)
            sw[0] = swfill(0, zeros_t[:, 0:HEAD_UNIT], HEAD_UNIT,
                           OUT_ELEMS // HEAD_UNIT - 1)
            nc.gpsimd.memset(zeros_t[:, HEAD_UNIT:4096], 0.0)
            nc.gpsimd.memset(zeros_t[:, 4096:BLK], 0.0)
            nc.vector.memset(ones_t[:, :], 1.0)
            for k in range(N_SH_ZEROS):
                sw[1 + N_SH_ONES + k] = swfill(
                    1 + N_SH_ONES + k, zeros_t[:, :], BLK, NBLOCKS - 1)

            def static_fill(eng, start, nblk):
                a, b = start * BLK, (start + nblk) * BLK
                t = ones_t if start < ZBASE else zeros_t
                return eng.dma_start(out[a:b], t[:, 0:nblk * BLK // 128])

            sync_f = [static_fill(nc.sync, s, n) for s, n in SYNC_FILLS]
            scal_f = [static_fill(nc.scalar, s, n) for s, n in SCAL_FILLS]

            # vals memsets before the ones-shaped gens: gpsimd stalls on
            # the ones_t memset at that point anyway. zeros-region scatter
            # columns write 1.0, ones-region columns 0.0.
            vals = pool.tile([128, N_SCAT], mybir.dt.float32)
            nc.gpsimd.memset(vals[:, 0:3], 1.0)
            nc.gpsimd.memset(vals[:, 3:N_SCAT], 0.0)

            for k in range(N_SH_ONES):
                sw[1 + k] = swfill(1 + k, ones_t[:, :], BLK, NBLOCKS - 1)

            statics = {s: f for (s, n), f in
                       zip(SYNC_FILLS, sync_f)} | \
                      {s: f for (s, n), f in zip(SCAL_FILLS, scal_f)}

            def covering(bmax_abs, lo_abs):
                """Static fills intersecting blocks [lo_abs, bmax_abs]."""
                res = []
                for (s, n) in SYNC_FILLS + SCAL_FILLS:
                    if s <= bmax_abs and s + n > lo_abs:
                        res.append(statics[s])
                return res

            # scatters in program = firing order: zeros cols fire mid-stream,
            # ones cols late, the tiny 24-point col last.
            for j, (reg, first, npts) in enumerate(SCAT_COLS):
                view = bass.AP(out.tensor, 0, [[1, 1], [1, 1]],
                               dep_tracking_offset=50000 + j)
                sc = nc.gpsimd.indirect_dma_start(
                    out=view,
                    out_offset=bass.IndirectOffsetOnAxis(
                        ap=ot[:, N_SW_FILLS + j:N_SW_FILLS + j + 1], axis=0),
                    in_=vals[:, j:j + 1], in_offset=None,
                    bounds_check=OUT_ELEMS - 1, oob_is_err=False)
                if reg == "o":
                    deps = sw[1:1 + N_SH_ONES] + covering(_col_bmax(j), 0)
                else:
                    deps = [sw[0]] + sw[1 + N_SH_ONES:] + covering(
                        ZBASE + _col_bmax(j), ZBASE)
                for f in deps:
                    add_dep_helper(sc.ins, f.ins,
                                   reason="scatter after covering fills")

    nc.compile()
    return nc


def _compute_indices(coord_v, lows, highs, nmc, L_):
    """Replicates reference.py exactly (same jax ops on the default device)
    so the floor/log10 bin boundaries match bit-for-bit."""
    import jax.numpy as jnp

    cv = jnp.asarray(np.asarray(coord_v, dtype=np.float32))
    n = cv.shape[1] // 3
    v10 = cv.at[:, 2::3].set(jnp.log10(cv[:, 2::3]))
    lo = jnp.tile(jnp.asarray(np.asarray(lows, dtype=np.float32)), n)
    hi = jnp.tile(jnp.asarray(np.asarray(highs, dtype=np.float32)), n)
    coord_grid = (v10 - lo) / (hi - lo)
    tr = coord_grid.reshape(-1, 3)
    x_i = jnp.floor(tr[:, 0] * L_).astype(jnp.int32)
    y_i = jnp.floor(tr[:, 1] * L_).astype(jnp.int32)
    m_i = jnp.floor(tr[:, 2] * nmc).astype(jnp.int32)
    return (np.asarray(x_i), np.asarray(y_i), np.asarray(m_i))


def _prepare_in_maps(coord_v, lows, highs, nmc, L):
    nmc = int(nmc)
    L_ = int(L)
    x_i, y_i, m_i = _compute_indices(coord_v, lows, highs, nmc, L_)
    n_batch = coord_v.shape[0]
    n = coord_v.shape[1] // 3
    b_i = np.repeat(np.arange(n_batch, dtype=np.int64), n)

    s_local = (b_i % BL).astype(np.int64)
    off_in_half = (m_i.astype(np.int64) * PLANE
                   + y_i.astype(np.int64) * L_ + x_i.astype(np.int64))
    ones_off = s_local * HALF + off_in_half
    zeros_off = REGION + ones_off

    in_maps = []
    pts_per_core = BL * n  # 320
    for c in range(NCORES):
        oc, zc = _LAYOUTS[STARVE_POS.get(c)]
        offs_np = np.full((128, NCOL), OOB, dtype=np.int32)
        # zeros head: 4KB units covering blocks HEAD_START..+15
        offs_np[:, 0] = (HEAD_START * (BLK // HEAD_UNIT)
                         + np.arange(128, dtype=np.int32))
        m = oc >= 0
        offs_np[:, 1:1 + N_SH_ONES][m] = oc[m].astype(np.int32)
        m = zc >= 0
        offs_np[:, 1 + N_SH_ONES:N_SW_FILLS][m] = zc[m].astype(np.int32)

        sel = slice(c * pts_per_core, (c + 1) * pts_per_core)
        po = np.sort(ones_off[sel])
        pz = np.sort(zeros_off[sel])
        for j, (reg, first, npts) in enumerate(SCAT_COLS):
            pts = (pz if reg == "z" else po)[first:first + npts]
            rows = _TINY_ROWS if npts == 24 else list(range(npts))
            offs_np[rows[:len(pts)], N_SW_FILLS + j] = pts.astype(np.int32)
            base = ZBASE if reg == "z" else 0
            assert (pts // BLK).max(initial=0) <= base + _col_bmax(j)
        in_maps.append({"offs": offs_np})
    return in_maps


def _run(in_maps, **kwargs):
    if "nc" not in _CACHE:
        _CACHE["nc"] = _build_nc()
    nc = _CACHE["nc"]
    from concourse.bass_utils import run_bass_kernel_spmd
    return run_bass_kernel_spmd(nc, in_maps, core_ids=list(range(NCORES)),
                                **kwargs)


def kernel(coord_v, lows, highs, nmc, L):
    nmc = int(nmc)
    L_ = int(L)
    assert nmc == NMC and L_ == globals()["L"], (nmc, L_)

    in_maps = _prepare_in_maps(coord_v, lows, highs, nmc, L_)
    res = _run(in_maps)
    parts = []
    for c in range(NCORES):
        o = res.results[c]["out"]
        ones = o[0:REGION].reshape(BL, NMC, L_, L_)
        zeros = o[REGION:].reshape(BL, NMC, L_, L_)
        parts.append(np.concatenate((ones, zeros), axis=1))
    return np.concatenate(parts, axis=0)


# revision 21
# speedup vs baseline: 1.1365x; 1.1365x over previous
"""Trainium2 Bass kernel for nn_CustomParameterTransform (scatter_memory).

Reference semantics: coord_v [256, 30] holds 10 (x, y, mass) triplets per
sample. Each triplet maps to integer grid indices (x_i, y_i, m_i); a one-hot
volume z [B, 16, 128, 128] is scattered (z[b, m, y, x] = 1) and the output is
concat(1-z, z) over the channel axis -> [256, 32, 128, 128] f32 (512 MB).

Strategy (8 NeuronCores, 32 samples/core, no cross-core comm): the output is
almost entirely constant, so the kernel is a pure HBM write stream (64 MB
per core) plus 640 one-element fixups per core.

Per-core output layout (host re-assembles): ones region [32 samples x 1 MB]
(the 1-z half: 1.0 except scatter points), then zeros region (the z half).

Fill plan, 2048 32KB blocks per core:
  - 52 MB static HWDGE fills (sync: most of ones; scalar: most of zeros)
    from constant SBUF tiles - every DMA engine gets exactly 104 blocks.
  - 12.5 MB early SWDGE indirect fills (gpsimd) whose 32KB blocks are
    addressed by a host-supplied per-core index tensor; descriptor slot
    rows map to fixed DMA engines (rows [4q,4q+4) -> engine (2q)%16 for
    q<16, else (2(q-16)+1)%16 - measured), and out-of-bounds indices are
    silently skipped, so the host shapes per-engine bytes per core.
  On this box one specific engine per even-numbered physical core
  intermittently runs ~20% slow (nc0/nc4 -> engine position 15, nc2/nc6 ->
  position 0; jax cores map to nc (4,5,6,7,2,3,0,1)). 104 blocks is the
  optimal share for a slow engine (104/21.3GB/s ~= 129.5/26.5GB/s), so on
  risky cores the host gives that engine no SWDGE blocks at all and spreads
  them over the other 15 engines; on healthy cores the layout is flat.
  Equalized finish ~157us vs ~197us for a flat layout with a slow engine,
  and the skew costs nothing when the engine is healthy.
  - 640 scatter fixups as 6 indirect-DMA columns (ones cols write 0.0,
    zeros cols 1.0), each depending only on the fills covering its
    address range so the last one fires right after the final fill.
"""

import numpy as np

B = 256
NSRC = 10
NMC = 16
L = 128
NCORES = 8
BL = B // NCORES            # 32 samples per core
PLANE = L * L               # 16384
HALF = NMC * PLANE          # 262144 elements per sample half (1 MB)
REGION = BL * HALF          # 8388608 elements per region (32 MB)
OUT_ELEMS = 2 * REGION      # 16777216 per core (64 MB)

BLK = 8192                  # elements per 32 KB fill block
NBLOCKS = OUT_ELEMS // BLK  # 2048
ZBASE = REGION // BLK       # first zeros-region block (1024)
HEAD_UNIT = 1024            # elements per 4 KB head-fill unit
HEAD_BLOCKS = 16            # head covers 512 KB = blocks 0..15

# SWDGE-shaped block ranges (absolute block ids). The zeros region is
# filled FIRST (mini fills bootstrap the stream from ~2.5us) and the ones
# region LAST, so six of the seven scatter columns fire mid-stream and only
# the tiny last ones-column trails the final fill.
ONES_SH_LO, ONES_SH_HI = 0, 192                  # 192 blocks
ZEROS_SH_LO, ZEROS_SH_HI = ZBASE + 128, ZBASE + 320  # 192 blocks
N_SH_ONES = 2               # shaped instrs per region (128 slots each)
N_SH_ZEROS = 2
N_SW_FILLS = N_SH_ZEROS + N_SH_ONES              # zeros instrs 0-1, ones 2-3

# Static HWDGE fills (start block, nblocks), in issue order per queue:
# both queues stream zeros minis, zeros statics, then ones; the 32-block
# ones tails are the very last fills.
MINI_BLOCKS = 16            # 0.5MB mini fills from the early mini tile
SYNC_FILLS = [(1024, 16), (1040, 16), (1056, 16), (1072, 16),
              (1344, 128), (1472, 128), (1600, 96),
              (192, 128), (320, 128), (448, 128), (576, 32)]
SCAL_FILLS = [(1088, 16), (1104, 16), (1120, 16), (1136, 16),
              (1696, 128), (1824, 128), (1952, 96),
              (608, 128), (736, 128), (864, 128), (992, 32)]

# Scatter columns: (region, first point, npoints). The last ones column is
# tiny (24 points) and confined to the rows of engines 2-4 so its
# completion (the only one after the final fill) costs ~3 engine-sem
# updates instead of 16.
SCAT_COLS = [("z", 0, 128), ("z", 128, 128), ("z", 256, 64),
             ("o", 0, 128), ("o", 128, 128), ("o", 256, 40), ("o", 296, 24)]
N_SCAT = len(SCAT_COLS)
NCOL = N_SW_FILLS + N_SCAT  # offs input columns

OOB = np.int32(0x7FFFFFF)

# jax core index -> engine position to starve (measured; absent = flat).
STARVE_POS = {0: 15, 2: 0, 4: 0, 6: 15}

_CACHE = {}


def _rows_of_pos(p):
    """The 8 descriptor-slot rows served by DMA engine position p."""
    if p % 2 == 0:
        q = p // 2
        return list(range(4 * q, 4 * q + 4)) + \
            list(range(4 * (q + 8), 4 * (q + 8) + 4))
    q = (p - 1) // 2
    return [64 + r for r in range(4 * q, 4 * q + 4)] + \
        [64 + r for r in range(4 * (q + 8), 4 * (q + 8) + 4)]


_POS_ROWS = [_rows_of_pos(p) for p in range(16)]


def _shaped_cols(starve_pos, block_ids, n_instr):
    """Assign shaped blocks to (instr, row) slots, engine-balanced.

    Returns int64 [128, n_instr] block ids (-1 = OOB slot)."""
    nb = len(block_ids)
    quota = [0] * 16
    if starve_pos is None:
        for p in range(16):
            quota[p] = nb // 16
        for p in range(nb % 16):
            quota[p] += 1
    else:
        healthy = [p for p in range(16) if p != starve_pos]
        for i, p in enumerate(healthy):
            quota[p] = nb // 15 + (1 if i < nb % 15 else 0)
    cols = np.full((128, n_instr), -1, dtype=np.int64)
    pos_it = 0
    for p in range(16):
        rows = _POS_ROWS[p]
        q = quota[p]
        assert q <= 8 * n_instr, (p, q)
        left = q
        for k in range(n_instr):
            t = min(8, left)
            for r in rows[:t]:
                cols[r, k] = block_ids[pos_it]
                pos_it += 1
            left -= t
    assert pos_it == nb, (pos_it, nb)
    return cols


def _class_layouts():
    out = {}
    for sp in (None, 0, 15):
        oc = _shaped_cols(sp, list(range(ONES_SH_LO, ONES_SH_HI)), N_SH_ONES)
        zc = _shaped_cols(sp, list(range(ZEROS_SH_LO, ZEROS_SH_HI)),
                          N_SH_ZEROS)
        out[sp] = (oc, zc)
    return out


_LAYOUTS = _class_layouts()


def _col_bmax(j):
    """Worst-case largest region-relative block id touched by scatter
    column j: points are exactly 10 per sample, address-sorted."""
    _, first, npts = SCAT_COLS[j]
    last_pt = first + npts - 1
    return 32 * (last_pt // 10 + 1) - 1


# Rows for the final tiny ones column: engines 2..4 (never the slow one).
_TINY_ROWS = sorted(_rows_of_pos(2) + _rows_of_pos(3) + _rows_of_pos(4))


def _build_nc():
    import concourse.bass as bass
    import concourse.tile as tile
    from concourse import bacc, mybir
    from concourse.tile_rust import add_dep_helper

    import types as _types
    from concourse.vector_clock import ScopedClock

    nc = bacc.Bacc("TRN2", target_bir_lowering=False, debug=False,
                   num_devices=NCORES)

    def _light_drain_and_barrier(self, tick_clock, wait_clock):
        """Replaces TileContext._drain_and_barrier. The stock epilogue is
        drain + two all-engine EVSEM butterfly barriers around the sem
        clear (~9 us). Requirements at kernel end: (1) all DMA completions
        observed, (2) sems cleared for NEFF re-execution, (3) the clear
        after every engine's last sem use. (1) is the sync drain's
        global-clock waits; (3) is a counting-sem join; (2) the ranged
        clear. The second barrier is unnecessary: a re-execution cannot
        start until every engine - including the clearing gpsimd - has
        ended."""
        nc_ = self.nc
        drain_inst = nc_.sync.drain()
        wait_clock.add_sem_waits(
            drain_inst.ins, ScopedClock({None: tick_clock.global_clock}))
        join = nc_.alloc_semaphore("tail_join")
        for eng in nc_.engines.values():
            if eng is not nc_.gpsimd:
                eng.sem_inc(join, 1)
        n_other = len(nc_.engines) - 1
        nc_.gpsimd.wait_ge(join, n_other)
        popped = nc_._tile_sem_poison_stack.pop()
        assert popped == self._sem_poison
        sems = list(self.sems.allocated().values())
        nc_.clear_and_free_semaphores(sems + [join])

    offs = nc.dram_tensor("offs", [128, NCOL], mybir.dt.int32,
                          kind="ExternalInput").ap()
    out = nc.dram_tensor("out", [OUT_ELEMS], mybir.dt.float32,
                         kind="ExternalOutput").ap()

    with tile.TileContext(nc) as tc:
        tc._drain_and_barrier = _types.MethodType(_light_drain_and_barrier, tc)
        with tc.tile_pool(name="src", bufs=1) as pool:
            zeros_t = pool.tile([128, BLK], mybir.dt.float32)
            ones_t = pool.tile([128, BLK], mybir.dt.float32)
            # Ramp: a [128,1024] zeros mini memsets fast on gpsimd (~1.5us)
            # and feeds 0.5MB HWDGE mini fills from ~2.5us; the big tiles
            # memset in halves on vector+gpsimd and take over from ~9us.
            zeros_mini = pool.tile([128, HEAD_UNIT], mybir.dt.float32)
            nc.gpsimd.memset(zeros_mini[:, :], 0.0)
            nc.vector.memset(zeros_t[0:64, :], 0.0)
            nc.gpsimd.memset(zeros_t[64:128, :], 0.0)

            ot = pool.tile([128, NCOL], mybir.dt.int32)
            nc.sync.dma_start(ot[:, :], offs[:, :])

            def swfill(j, in_ap, bound):
                view = bass.AP(out.tensor, 0, [[BLK, 1], [1, BLK]],
                               dep_tracking_offset=j * BLK)
                return nc.gpsimd.indirect_dma_start(
                    out=view,
                    out_offset=bass.IndirectOffsetOnAxis(
                        ap=ot[:, j:j + 1], axis=0),
                    in_=in_ap, in_offset=None,
                    bounds_check=bound, oob_is_err=False)

            sw = [None] * N_SW_FILLS
            for k in range(N_SH_ZEROS):
                sw[k] = swfill(k, zeros_t[:, :], NBLOCKS - 1)

            nc.vector.memset(ones_t[0:64, :], 1.0)
            nc.gpsimd.memset(ones_t[64:128, :], 1.0)

            # zeros-region scatter columns write 1.0, ones-region 0.0
            vals = pool.tile([128, N_SCAT], mybir.dt.float32)
            nc.gpsimd.memset(vals[:, 0:3], 1.0)
            nc.gpsimd.memset(vals[:, 3:N_SCAT], 0.0)

            for k in range(N_SH_ONES):
                sw[N_SH_ZEROS + k] = swfill(N_SH_ZEROS + k, ones_t[:, :],
                                            NBLOCKS - 1)

            def static_fill(eng, start, nblk):
                a, b = start * BLK, (start + nblk) * BLK
                if nblk == MINI_BLOCKS:
                    return eng.dma_start(out[a:b], zeros_mini[:, :])
                t = ones_t if start < ZBASE else zeros_t
                return eng.dma_start(out[a:b], t[:, 0:nblk * BLK // 128])

            sync_f = [static_fill(nc.sync, s, n) for s, n in SYNC_FILLS]
            scal_f = [static_fill(nc.scalar, s, n) for s, n in SCAL_FILLS]

            statics = {s: f for (s, n), f in
                       zip(SYNC_FILLS, sync_f)} | \
                      {s: f for (s, n), f in zip(SCAL_FILLS, scal_f)}

            def covering(bmax_abs, lo_abs):
                """Static fills intersecting blocks [lo_abs, bmax_abs]."""
                res = []
                for (s, n) in SYNC_FILLS + SCAL_FILLS:
                    if s <= bmax_abs and s + n > lo_abs:
                        res.append(statics[s])
                return res

            # scatters in program = firing order: zeros cols fire mid-stream,
            # ones cols late, the tiny 24-point col last.
            for j, (reg, first, npts) in enumerate(SCAT_COLS):
                view = bass.AP(out.tensor, 0, [[1, 1], [1, 1]],
                               dep_tracking_offset=50000 + j)
                sc = nc.gpsimd.indirect_dma_start(
                    out=view,
                    out_offset=bass.IndirectOffsetOnAxis(
                        ap=ot[:, N_SW_FILLS + j:N_SW_FILLS + j + 1], axis=0),
                    in_=vals[:, j:j + 1], in_offset=None,
                    bounds_check=OUT_ELEMS - 1, oob_is_err=False)
                if reg == "o":
                    deps = sw[N_SH_ZEROS:] + covering(_col_bmax(j), 0)
                else:
                    deps = sw[:N_SH_ZEROS] + covering(
                        ZBASE + _col_bmax(j), ZBASE)
                for f in deps:
                    add_dep_helper(sc.ins, f.ins,
                                   reason="scatter after covering fills")

    nc.compile()
    return nc


def _compute_indices(coord_v, lows, highs, nmc, L_):
    """Replicates reference.py exactly (same jax ops on the default device)
    so the floor/log10 bin boundaries match bit-for-bit."""
    import jax.numpy as jnp

    cv = jnp.asarray(np.asarray(coord_v, dtype=np.float32))
    n = cv.shape[1] // 3
    v10 = cv.at[:, 2::3].set(jnp.log10(cv[:, 2::3]))
    lo = jnp.tile(jnp.asarray(np.asarray(lows, dtype=np.float32)), n)
    hi = jnp.tile(jnp.asarray(np.asarray(highs, dtype=np.float32)), n)
    coord_grid = (v10 - lo) / (hi - lo)
    tr = coord_grid.reshape(-1, 3)
    x_i = jnp.floor(tr[:, 0] * L_).astype(jnp.int32)
    y_i = jnp.floor(tr[:, 1] * L_).astype(jnp.int32)
    m_i = jnp.floor(tr[:, 2] * nmc).astype(jnp.int32)
    return (np.asarray(x_i), np.asarray(y_i), np.asarray(m_i))


def _prepare_in_maps(coord_v, lows, highs, nmc, L):
    nmc = int(nmc)
    L_ = int(L)
    x_i, y_i, m_i = _compute_indices(coord_v, lows, highs, nmc, L_)
    n_batch = coord_v.shape[0]
    n = coord_v.shape[1] // 3
    b_i = np.repeat(np.arange(n_batch, dtype=np.int64), n)

    s_local = (b_i % BL).astype(np.int64)
    off_in_half = (m_i.astype(np.int64) * PLANE
                   + y_i.astype(np.int64) * L_ + x_i.astype(np.int64))
    ones_off = s_local * HALF + off_in_half
    zeros_off = REGION + ones_off

    in_maps = []
    pts_per_core = BL * n  # 320
    for c in range(NCORES):
        oc, zc = _LAYOUTS[STARVE_POS.get(c)]
        offs_np = np.full((128, NCOL), OOB, dtype=np.int32)
        m = zc >= 0
        offs_np[:, 0:N_SH_ZEROS][m] = zc[m].astype(np.int32)
        m = oc >= 0
        offs_np[:, N_SH_ZEROS:N_SW_FILLS][m] = oc[m].astype(np.int32)

        sel = slice(c * pts_per_core, (c + 1) * pts_per_core)
        po = np.sort(ones_off[sel])
        pz = np.sort(zeros_off[sel])
        for j, (reg, first, npts) in enumerate(SCAT_COLS):
            pts = (pz if reg == "z" else po)[first:first + npts]
            rows = _TINY_ROWS if npts == 24 else list(range(npts))
            offs_np[rows[:len(pts)], N_SW_FILLS + j] = pts.astype(np.int32)
            base = ZBASE if reg == "z" else 0
            assert (pts // BLK).max(initial=0) <= base + _col_bmax(j)
        in_maps.append({"offs": offs_np})
    return in_maps


def _run(in_maps, **kwargs):
    if "nc" not in _CACHE:
        _CACHE["nc"] = _build_nc()
    nc = _CACHE["nc"]
    from concourse.bass_utils import run_bass_kernel_spmd
    return run_bass_kernel_spmd(nc, in_maps, core_ids=list(range(NCORES)),
                                **kwargs)


def kernel(coord_v, lows, highs, nmc, L):
    nmc = int(nmc)
    L_ = int(L)
    assert nmc == NMC and L_ == globals()["L"], (nmc, L_)

    in_maps = _prepare_in_maps(coord_v, lows, highs, nmc, L_)
    res = _run(in_maps)
    parts = []
    for c in range(NCORES):
        o = res.results[c]["out"]
        ones = o[0:REGION].reshape(BL, NMC, L_, L_)
        zeros = o[REGION:].reshape(BL, NMC, L_, L_)
        parts.append(np.concatenate((ones, zeros), axis=1))
    return np.concatenate(parts, axis=0)
